# revision 31
# baseline (speedup 1.0000x reference)
"""Trainium2 Bass kernel for nn_DecoderForLarge (sparse_attention).

Math (per batch b):
  probs = softmax(10*tanh(a*final_q @ M @ emb.T - dist/sqrt(2)) + mask)
where the multi-head mean collapses the attention into one H-dim bilinear
form with M := Wq.T @ Wk, and final_q folds q_last/q_first/q_graph/q_visited
into three HxH matrices A,B,C (precomputed once on device).

v3 strategy (baseline v2 was latency-bound at 184us with every engine
<50% busy, 10 ACT table loads, 63us of HAM throttle):
  - G-packing: per-core 4 batches x G=200 groups are packed into 7 tiles of
    128 rows (per-batch G padded to 224 so batch boundaries inside a tile
    land on 32-aligned partitions, which PE matmul output APs require).
    22% less ACT/DVE/elementwise work than the old 8-tile layout.
  - distances: ONE shared [16,NPAD] fp32r table (rows 0-7: -centered
    coords b-major, rows 8-15: squares); per-tile lhs [16,128] is zero
    except each column's own batch rows (coords via tiny transpose-DMA,
    0.5 in the square rows so the matmul emits 0.5*d2 directly).
    DVE fuses clamp+bias in one tensor_scalar (max then add).
  - visited mask folded as +300 into the distance tile (ds_m), so
    tanh(score - ds_m) = -1 at visited nodes and exp(10*tanh) ~ 0:
    no separate mask add on the softmax path, exp uses scale=10.
  - ACT queue order: all 7 sqrts, then all tanh/exp (tanh+exp share the
    exp_and_others table set) -> exactly 2 table loads.
  - all transposed layouts (emb.T, vis.T) come pre-transposed from the
    host repack; zero on-device DMA transposes of big tiles.
  - f16 everywhere on the score path (as v2), fp32r distances.
Sharding: data-parallel over batch B=32 -> 8 cores x 4 batches.
"""
import sys

sys.path.insert(0, "/opt/trn_rl_repo")

import numpy as np

import concourse.bass as bass
import concourse.tile as tile
from concourse import mybir
from concourse.masks import make_identity


def _ensure_axon_hooks():
    """The image's antenv may lack axon_hooks, which bass_utils imports
    when trace=True under axon. Inject it and register the real NTFF
    profiling hook if the injected .so supports it."""
    try:
        import antenv.axon_hooks  # noqa: F401
        return
    except ImportError:
        pass
    import types
    import antenv

    mod = types.ModuleType("antenv.axon_hooks")
    mod._hook = None
    mod.set_axon_ntff_profile_hook = lambda h: setattr(mod, "_hook", h)
    mod.get_axon_ntff_profile_hook = lambda: mod._hook
    sys.modules["antenv.axon_hooks"] = mod
    antenv.axon_hooks = mod
    try:
        from trn_agent_boot.trn_boot import _ntff_profile_via_ctypes
        mod._hook = _ntff_profile_via_ctypes("/opt/axon/libaxon_pjrt.so")
    except Exception:
        mod._hook = None


_ensure_axon_hooks()

F32 = mybir.dt.float32
F32R = mybir.dt.float32r
F16 = mybir.dt.float16
I32 = mybir.dt.int32

B, N, G, H, NH, D = 32, 2000, 200, 128, 8, 2
NCORES = 8
BPC = B // NCORES          # batches per core
NPAD = 2048                # N padded to 16*128
NCH = NPAD // 128          # n column chunks
GPAD = 224                 # per-batch G padded to a multiple of 32
KH = 72                    # dist-matmul K: 0-7 chat, 32-39 hi, 64-71 lo
GONE = GPAD + 1            # vis.T cols (zero-padded to GPAD) + ones col
NT = BPC * GPAD // 128     # 7 packed g-tiles per core
ALPHA = 1.0 / (NH * np.sqrt(np.float32(H)))   # head-mean * 1/sqrt(H)
AF = mybir.ActivationFunctionType
OP = mybir.AluOpType
W_NAMES = ["Wq_graph", "Wq_first", "Wq_last", "Wq", "W_visited", "Wk"]

# segments: per packed tile, (row0, nrows, batch) pieces; GPAD=224 makes all
# row0 multiples of 32 (PE matmul out base-partition constraint)
def _build_segs():
    segs_all = []
    for t in range(NT):
        segs = []
        r = 0
        while r < 128:
            g_global = 128 * t + r
            b = g_global // GPAD
            seg_len = min(128 - r, GPAD * (b + 1) - g_global)
            segs.append((r, seg_len, b))
            r += seg_len
        segs_all.append(segs)
    return segs_all


SEGS = _build_segs()
# tiles whose last-needed batch b: issue vis(b) before this tile's qsum
TILE_MAX_B = [max(b for (_, _, b) in SEGS[t]) for t in range(NT)]


def _sub_segs(segs):
    """Split segments so each piece is a legal PE matmul output slab:
    base partition 0 (any size) or 32/64/96 with col-tile-size limits."""
    out = []
    for (r0, ln, b) in segs:
        if r0 == 0:
            out.append((0, ln, b))
            continue
        r, rem = r0, ln
        while rem > 0:
            cap = {32: 32, 64: 64, 96: 32}[r]
            take = min(rem, cap)
            out.append((r, take, b))
            r += take
            rem -= take
    return out


SUBSEGS = [_sub_segs(s) for s in SEGS]


def build_nc() -> bass.Bass:
    nc = bass.Bass()

    embc_d = nc.dram_tensor("embc", [BPC, NPAD, H + D], F16,
                            kind="ExternalInput")
    embt_d = nc.dram_tensor("embt", [BPC, H, NPAD], F16, kind="ExternalInput")
    vist_d = nc.dram_tensor("vist", [BPC, NPAD, GONE], F16,
                            kind="ExternalInput")
    gnm_d = nc.dram_tensor("gnm", [NT, 128, NPAD], F16, kind="ExternalInput")
    coordt_d = nc.dram_tensor("coordt", [BPC * D, NPAD], F32,
                              kind="ExternalInput")
    lastn_d = nc.dram_tensor("lastn", [128, NT], I32, kind="ExternalInput")
    lhalf_d = nc.dram_tensor("lhalf", [KH - BPC * D, NT * 128], F16,
                             kind="ExternalInput")
    w6_d = nc.dram_tensor("w6", [H, 6 * H], F32, kind="ExternalInput")
    out_d = nc.dram_tensor("probs", [NT * 128, N], F16, kind="ExternalOutput")

    embc_flat = embc_d.rearrange("b n w -> (b n) w")

    with tile.TileContext(nc) as tc:
        with (
            tc.tile_pool(name="consts", bufs=1) as consts,
            tc.tile_pool(name="inb", bufs=1) as inb,        # big inputs
            tc.tile_pool(name="gpool", bufs=1) as gpool,    # gnm tiles (7)
            tc.tile_pool(name="dsm", bufs=1) as dsmp,       # ds_m tiles (7)
            tc.tile_pool(name="p1s", bufs=2) as p1s,        # phase-1 smalls
            tc.tile_pool(name="csp", bufs=1) as csp,        # coord setup
            tc.tile_pool(name="dsp", bufs=2) as dsp,        # sqrt out
            tc.tile_pool(name="ew", bufs=2) as ew,          # th/e/pr
            tc.tile_pool(name="sm", bufs=2) as sm,          # small scratch
            tc.tile_pool(name="pp_s", bufs=1, space="PSUM") as pp_s,  # 4 bk
            tc.tile_pool(name="pp_d", bufs=2, space="PSUM") as pp_d,  # 1bk x2
            tc.tile_pool(name="pp_t", bufs=2, space="PSUM") as pp_t,  # 1bk x2
        ):
            # ---- sync queue: small critical loads, then big streams ------
            lastn = consts.tile([128, NT], I32)
            nc.sync.dma_start(out=lastn, in_=lastn_d[:, :])
            ct_raw = csp.tile([BPC * D, NPAD], F32)
            nc.sync.dma_start(out=ct_raw, in_=coordt_d[:, :])
            embn, embt, vist = {}, {}, {}

            def load_embc(b):
                embn[b] = inb.tile([128, NCH, H], F16, tag=f"embn{b}",
                                   name=f"embn{b}")
                nc.sync.dma_start(
                    out=embn[b],
                    in_=embc_d[b, :, 0:H].rearrange("(p c) h -> p c h",
                                                    c=NCH))

            def load_vist(b):
                vist[b] = inb.tile([128, NCH, GONE], F16, tag=f"vist{b}",
                                   name=f"vist{b}")
                nc.sync.dma_start(
                    out=vist[b],
                    in_=vist_d[b].rearrange("(p c) g -> p c g", c=NCH))

            def load_embt(b, eng):
                embt[b] = inb.tile([128, NCH, 128], F16, tag=f"embt{b}",
                                   name=f"embt{b}")
                eng.dma_start(
                    out=embt[b],
                    in_=embt_d[b].rearrange("h (c n) -> h c n", n=128))

            load_embc(0)
            load_vist(0)
            w6 = consts.tile([H, 6 * H], F32)
            nc.sync.dma_start(out=w6, in_=w6_d[:, :])
            load_embc(1)
            load_vist(1)
            load_embt(0, nc.sync)
            load_embc(2)
            load_vist(2)
            load_embt(1, nc.sync)
            load_embc(3)
            load_vist(3)

            # ---- gpsimd queue: iota, lhs static rows, gathers, segs ------
            ident16 = consts.tile([128, 128], F16)
            make_identity(nc, ident16)
            GRP = {0: (0, 2), 1: (2, NT)}
            lhs_g = {}
            for g, (t0, t1) in GRP.items():
                w = (t1 - t0) * 128
                lhs_g[g] = consts.tile([KH, w], F16,
                                       tag=f"lhs_g{g}", name=f"lhs_g{g}")
                nc.gpsimd.memset(lhs_g[g][0:BPC * D], 0.0)
                nc.gpsimd.dma_start(
                    out=lhs_g[g][BPC * D:KH],
                    in_=lhalf_d[:, t0 * 128:t1 * 128])

            def lhs_of(t):
                g = 0 if t < 2 else 1
                o = t * 128 - GRP[g][0] * 128
                return lhs_g[g][:, o:o + 128]

            lne_all = csp.tile([128, NT, H + D], F16)

            def do_gathers(t0, t1):
                for t in range(t0, t1):
                    nc.gpsimd.indirect_dma_start(
                        out=lne_all[:, t, :], out_offset=None, in_=embc_flat,
                        in_offset=bass.IndirectOffsetOnAxis(
                            ap=lastn[:, t:t + 1], axis=0))

            # ---- coord table cs24, built in 512-col chunks across ACT+DVE
            # rows 0-7: chat = -(c-0.5) f16; rows 8-15: f16-hi of chat^2;
            # rows 16-23: f16-lo residual -> the K=24 f16 matmul emits
            # 0.5*d2 - 0.5*r2 exactly to ~1e-7 at f16 speed.
            # table sections at partitions 0 (chat), 32 (hi), 64 (lo) so
            # compute engines can write them (32-aligned bases); the K=72
            # matmul's zero lhs rows make the gap rows inert (zeroed once)
            cs72 = consts.tile([KH, NPAD], F16)
            nc.gpsimd.memset(cs72, 0.0)
            tsq = csp.tile([BPC * D, NPAD], F32)
            thi = csp.tile([BPC * D, NPAD], F16)

            def table_chunk(q):
                sl = slice(q * 512, (q + 1) * 512)
                nc.vector.tensor_scalar(out=cs72[0:BPC * D, sl],
                                        in0=ct_raw[:, sl], scalar1=-1.0,
                                        scalar2=0.5, op0=OP.mult, op1=OP.add)
                nc.scalar.activation(out=tsq[:, sl], in_=cs72[0:BPC * D, sl],
                                     func=AF.Square)
                nc.vector.tensor_copy(out=thi[:, sl], in_=tsq[:, sl])
                nc.scalar.activation(out=cs72[32:32 + BPC * D, sl],
                                     in_=thi[:, sl], func=AF.Copy)
                nc.vector.tensor_tensor(out=cs72[64:64 + BPC * D, sl],
                                        in0=tsq[:, sl], in1=thi[:, sl],
                                        op=OP.subtract)

            for q in range(4):
                table_chunk(q)

            # batched last-node prep per group
            lcc_all = p1s.tile([128, NT, D], F16, tag="lcc_all")
            bias_all = consts.tile([128, NT], F32)

            def do_prep(t0, t1):
                nt = t1 - t0
                nc.vector.tensor_scalar(out=lcc_all[:, t0:t1, :],
                                        in0=lne_all[:, t0:t1, H:H + D],
                                        scalar1=-0.5, scalar2=None,
                                        op0=OP.add)
                sq = p1s.tile([128, NT, D], F32, tag="sq_all",
                              name=f"sq_{t0}")
                nc.vector.tensor_tensor(out=sq[:, t0:t1, :],
                                        in0=lcc_all[:, t0:t1, :],
                                        in1=lcc_all[:, t0:t1, :],
                                        op=OP.mult)
                r2 = p1s.tile([128, NT], F32, tag="r2_all", name=f"r2_{t0}")
                nc.vector.tensor_tensor(out=r2[:, t0:t1],
                                        in0=sq[:, t0:t1, 0],
                                        in1=sq[:, t0:t1, 1], op=OP.add)
                nc.vector.tensor_scalar(out=bias_all[:, t0:t1],
                                        in0=r2[:, t0:t1], scalar1=0.5,
                                        scalar2=1e-6, op0=OP.mult,
                                        op1=OP.add)
                lcT_p = pp_t.tile([NT * D, 128], F16, tag="pt",
                                  name=f"lcT_p{t0}")
                nc.tensor.transpose(
                    lcT_p[0:nt * D, :],
                    lcc_all[:, t0:t1, :].rearrange("p t d -> p (t d)"),
                    ident16)
                lcT_s = p1s.tile([NT * D, 128], F16, tag=f"lcT_s{t0}",
                                 name=f"lcT_s{t0}")
                nc.vector.tensor_copy(out=lcT_s[0:nt * D, :],
                                      in_=lcT_p[0:nt * D, :])
                for t in range(t0, t1):
                    for (r0, ln, b) in SEGS[t]:
                        nc.gpsimd.dma_start(
                            out=lhs_of(t)[2 * b:2 * b + 2, r0:r0 + ln],
                            in_=lcT_s[2 * (t - t0):2 * (t - t0) + 2,
                                      r0:r0 + ln])

            do_gathers(0, 2)
            do_prep(0, 2)
            do_gathers(2, NT)

            # ---- scalar queue: gnm 0-2 early (needed by ds_m t0-2) ------
            gnm = {}

            def load_gnm(t, eng):
                gnm[t] = gpool.tile([128, NPAD], F16, tag=f"gnm{t}",
                                    name=f"gnm{t}")
                eng.dma_start(out=gnm[t], in_=gnm_d[t])

            for t in range(3):
                load_gnm(t, nc.scalar)

            # ---------------- phase 1: dist matmuls + everything else ----
            vembt = consts.tile([H, BPC, GONE], F16)
            qg_s = consts.tile([H, BPC], F32)
            lnetT = consts.tile([H, NT, 128], F16)
            ds_all = {}
            ds_m = {}

            def do_vis(b):
                vemb_p = pp_t.tile([H, GONE], F32, tag="pt", name=f"vemb{b}")
                for c in range(NCH):
                    nc.tensor.matmul(vemb_p, embn[b][:, c, :],
                                     vist[b][:, c, :],
                                     start=(c == 0), stop=(c == NCH - 1))
                nc.vector.tensor_copy(out=vembt[:, b, :], in_=vemb_p)

            def do_dist(t):
                ds = dsp.tile([128, NPAD], F16, tag="ds")
                for q in range(4):
                    d2_p = pp_d.tile([128, 512], F32, tag="pd", name="d2_p")
                    nc.tensor.matmul(d2_p, lhs_of(t),
                                     cs72[:, q * 512:(q + 1) * 512],
                                     start=True, stop=True)
                    nc.scalar.activation(
                        out=ds[:, q * 512:(q + 1) * 512], in_=d2_p,
                        func=AF.Sqrt, bias=bias_all[:, t:t + 1])
                ds_all[t] = ds

            def do_dsm(t):
                dm = dsmp.tile([128, NPAD], F16, tag=f"dsm{t}",
                               name=f"dsm{t}")
                nc.vector.tensor_tensor(out=dm, in0=ds_all[t], in1=gnm[t],
                                        op=OP.add)
                ds_m[t] = dm

            do_dist(0)
            do_vis(0)
            do_dsm(0)
            do_dist(1)
            do_vis(1)
            do_dsm(1)
            do_prep(2, NT)          # group-B transposes + seg DMAs
            load_gnm(3, nc.gpsimd)
            load_gnm(4, nc.gpsimd)
            do_dist(2)
            do_dsm(2)

            # weight chain (tiny, runs inside the sqrt window)
            negi16 = consts.tile([128, 128], F16)
            nc.vector.tensor_scalar(out=negi16, in0=ident16, scalar1=-1.0,
                                    scalar2=None, op0=OP.mult)
            w_s = {n: w6[:, i * H:(i + 1) * H] for i, n in enumerate(W_NAMES)}
            wlf = consts.tile([H, H], F32)
            nc.vector.tensor_tensor(out=wlf, in0=w_s["Wq_last"],
                                    in1=w_s["Wq_first"], op=OP.add)
            mt_p = pp_t.tile([H, H], F32, tag="pt", name="mt_p")
            nc.tensor.matmul(mt_p, w_s["Wq"], w_s["Wk"], start=True, stop=True)
            mt_s = consts.tile([H, H], F32)
            nc.vector.tensor_copy(out=mt_s, in_=mt_p)
            abc = {}
            for nm, lhs, scale in (
                ("A", wlf, ALPHA),
                ("Bm", w_s["Wq_graph"], ALPHA / N),
                ("C", w_s["W_visited"], ALPHA / N),
            ):
                pp = pp_t.tile([H, H], F32, tag="pt", name=f"abc_p_{nm}")
                nc.tensor.matmul(pp, lhs, mt_s, start=True, stop=True)
                abc[nm] = consts.tile([H, H], F16, tag=f"abc_{nm}",
                                      name=f"abc_{nm}")
                nc.vector.tensor_scalar(out=abc[nm], in0=pp,
                                        scalar1=float(scale),
                                        scalar2=None, op0=OP.mult)

            do_dist(3)
            do_vis(2)
            do_dsm(3)
            load_gnm(5, nc.gpsimd)
            load_gnm(6, nc.gpsimd)
            load_embt(2, nc.gpsimd)
            load_embt(3, nc.gpsimd)
            do_dist(4)
            do_vis(3)
            do_dsm(4)
            do_dist(5)
            do_dsm(5)
            do_dist(6)
            do_dsm(6)

            # q_graph per batch + last-node transposes
            for b in range(BPC):
                qg_p = pp_t.tile([H, 1], F32, tag="pt", name=f"qg{b}")
                nc.tensor.matmul(qg_p, abc["Bm"], vembt[:, b, GPAD:GPAD + 1],
                                 start=True, stop=True)
                nc.vector.tensor_copy(out=qg_s[:, b:b + 1], in_=qg_p)
            for t in range(NT):
                lnet_p = pp_t.tile([H, 128], F16, tag="pt", name=f"lnp{t}")
                nc.tensor.transpose(lnet_p, lne_all[:, t, 0:H], ident16)
                nc.vector.tensor_copy(out=lnetT[:, t, :], in_=lnet_p)

            # ---------------- phase 2+3 per tile --------------------------
            qsumt = {}
            for t in range(NT):
                qsum_p = pp_t.tile([H, 128], F32, tag="pt", name=f"qsp{t}")
                nc.tensor.matmul(qsum_p, abc["A"], lnetT[:, t, :],
                                 start=True, stop=False)
                for i, (r0, ln, b) in enumerate(SEGS[t]):
                    gg = 128 * t + r0 - GPAD * b
                    nc.tensor.matmul(qsum_p[:, r0:r0 + ln], abc["C"],
                                     vembt[:, b, gg:gg + ln],
                                     start=False, stop=(i == len(SEGS[t]) - 1))
                qt = sm.tile([H, 128], F16, tag="qsumt", name=f"qsumt{t}")
                for (r0, ln, b) in SEGS[t]:
                    nc.vector.tensor_scalar(out=qt[:, r0:r0 + ln],
                                            in0=qsum_p[:, r0:r0 + ln],
                                            scalar1=qg_s[:, b:b + 1],
                                            scalar2=None, op0=OP.add)
                qsumt[t] = qt

                th = ew.tile([128, NPAD], F16, tag="th")
                sp = pp_s.tile([128, NPAD], F32, tag="ps", name="sp")
                for q in range(4):
                    nc.tensor.matmul(
                        sp[:, q * 512:(q + 1) * 512], negi16,
                        ds_m[t][:, q * 512:(q + 1) * 512],
                        start=True, stop=False)
                embt2 = {b: embt[b].rearrange("p c n -> p (c n)")
                         for b in set(b for (_, _, b) in SEGS[t])}
                for i, (r0, ln, b) in enumerate(SUBSEGS[t]):
                    last = i == len(SUBSEGS[t]) - 1
                    tp = None if r0 == 0 else (0, r0)
                    for q in range(4):
                        nc.tensor.matmul(
                            sp[r0:r0 + ln, q * 512:(q + 1) * 512],
                            qsumt[t][:, r0:r0 + ln],
                            embt2[b][:, q * 512:(q + 1) * 512],
                            start=False, stop=last, tile_position=tp)
                nc.scalar.activation(out=th, in_=sp, func=AF.Tanh)
                e = ew.tile([128, NPAD], F16, tag="e")
                esum = sm.tile([128, 1], F32, tag="esum")
                nc.scalar.activation(out=e, in_=th, func=AF.Exp, scale=10.0,
                                     accum_out=esum[:, :])
                nc.vector.reciprocal(out=esum, in_=esum)
                pr = ew.tile([128, NPAD], F16, tag="pr")
                nc.vector.tensor_scalar(out=pr, in0=e, scalar1=esum[:, :],
                                        scalar2=None, op0=OP.mult)
                nc.sync.dma_start(out=out_d[t * 128:(t + 1) * 128, :],
                                  in_=pr[:, 0:N])
    return nc


def _split_multi_waits(bir: bytes, max_inline: int = 1) -> bytes:
    """This walrus build only accepts one inline sync-wait per instruction;
    Tile inlines many. Split extras into standalone EventSemaphore waits
    (same engine, immediately before), which is exactly the raw-bass form."""
    import orjson

    j = orjson.loads(bir)
    ctr = 0
    for fn in j["functions"]:
        for blk in fn["blocks"]:
            insts = blk.get("instructions")
            if not insts:
                continue
            out = []
            for inst in insts:
                si = inst.get("sync_info")
                waits = (si or {}).get("on_wait") or []
                if len(waits) > max_inline:
                    for w in waits[:-max_inline]:
                        ctr += 1
                        out.append({
                            "name": f"SW-{ctr}",
                            "opcode": "EventSemaphore",
                            "engine": inst["engine"],
                            "ins": [],
                            "outs": [],
                            "sync_info": {"on_wait": [w], "on_update": []},
                        })
                    si["on_wait"] = waits[-max_inline:]
                out.append(inst)
            blk["instructions"] = out
    return orjson.dumps(j)


_NC = None


def _get_nc():
    global _NC
    if _NC is None:
        _NC = build_nc()
        transformed = _split_multi_waits(_NC.to_json_bytes())
        _NC.to_json_bytes = lambda: transformed
    return _NC


def make_in_maps(embeddings, coordinates, last_node, group_ninf_mask,
                 Wq_graph, Wq_first, Wq_last, Wq, W_visited, Wk):
    """Shard + pad + repack full inputs into 8 per-core input maps."""
    emb_p = np.zeros((B, NPAD, H + D), np.float16)
    emb_p[:, :N, :H] = embeddings.astype(np.float16)
    emb_p[:, :N, H:] = coordinates.astype(np.float16)
    embt_p = np.ascontiguousarray(
        emb_p[:, :, :H].transpose(0, 2, 1))  # (B, H, NPAD)
    coord_p = np.zeros((B, NPAD, D), np.float32)
    coord_p[:, :N] = coordinates
    # (B, D, NPAD) -> per core [8, NPAD] -> pre-tiled [128, NPAD/16]
    coordt = np.ascontiguousarray(coord_p.transpose(0, 2, 1))

    vis = np.isneginf(group_ninf_mask)                         # (B, G, N)
    vist_p = np.zeros((B, NPAD, GONE), np.float16)
    vist_p[:, :N, :G] = vis.transpose(0, 2, 1)
    vist_p[:, :, GPAD] = 1.0                                      # colsum ones

    gnm300 = np.full((B, GPAD, NPAD), 300.0, np.float16)
    gnm300[:, :G, :N] = vis.astype(np.float16) * np.float16(300.0)

    lastn = np.zeros((B, GPAD), np.int32)
    lastn[:, :G] = np.asarray(last_node).astype(np.int64).astype(np.int32)
    # pre-offset indices into the per-core flattened (BPC*NPAD, .) source
    lastn += (np.arange(B, dtype=np.int32) % BPC)[:, None] * NPAD

    # static 0.5 pattern for the lhs square rows (rows 8-23 of lhs_all):
    # row r (0..15) -> lhs row 8+r, batch (r % 8)//2; 0.5 where the packed
    # g-column belongs to that batch
    col_b = np.arange(NT * 128) // GPAD
    lhalf = np.zeros((KH - BPC * D, NT * 128), np.float16)
    for sec in (32, 64):                    # hi and lo 0.5-pattern rows
        for b in range(BPC):
            for dd in range(D):
                lhalf[sec + 2 * b + dd - BPC * D,
                      col_b == b] = 0.5

    w6 = np.ascontiguousarray(np.concatenate(
        [np.asarray(w, np.float32) for w in
         (Wq_graph, Wq_first, Wq_last, Wq, W_visited, Wk)], axis=1))

    in_maps = []
    for i in range(NCORES):
        sl = slice(i * BPC, (i + 1) * BPC)
        lpk = lastn[sl].reshape(NT, 128)          # packed rows, tile-major
        m = {
            "embc": np.ascontiguousarray(emb_p[sl]),
            "embt": np.ascontiguousarray(embt_p[sl]),
            "vist": np.ascontiguousarray(vist_p[sl]),
            "gnm": np.ascontiguousarray(
                gnm300[sl].reshape(NT, 128, NPAD)),
            "coordt": np.ascontiguousarray(
                coordt[sl].reshape(BPC * D, NPAD)),
            "lastn": np.ascontiguousarray(lpk.T),  # [128, NT]
            "lhalf": lhalf,
            "w6": w6,
        }
        in_maps.append(m)
    return in_maps


def kernel(embeddings, coordinates, last_node, group_ninf_mask, S,
           Wq_graph, Wq_first, Wq_last, Wq, W_visited, Wk, **run_kwargs):
    from concourse.bass_utils import run_bass_kernel_spmd

    nc = _get_nc()
    in_maps = make_in_maps(
        np.asarray(embeddings), np.asarray(coordinates), np.asarray(last_node),
        np.asarray(group_ninf_mask), np.asarray(Wq_graph), np.asarray(Wq_first),
        np.asarray(Wq_last), np.asarray(Wq), np.asarray(W_visited),
        np.asarray(Wk))
    res = run_bass_kernel_spmd(nc, in_maps, core_ids=list(range(NCORES)),
                               **run_kwargs)
    # unpack: [NT*128, N] -> (BPC, GPAD, N) -> (BPC, G, N)
    outs = []
    for r in res.results:
        o = r["probs"].reshape(BPC, GPAD, N)[:, :G, :]
        outs.append(o)
    out = np.concatenate(outs, axis=0)
    kernel.last_results = res
    return out.astype(np.float32)


# revision 32
# speedup vs baseline: 1.0611x; 1.0611x over previous
"""Trainium2 Bass kernel for nn_DecoderForLarge (sparse_attention).

Math (per batch b):
  probs = softmax(10*tanh(a*final_q @ M @ emb.T - dist/sqrt(2)) + mask)
where the multi-head mean collapses the attention into one H-dim bilinear
form with M := Wq.T @ Wk, and final_q folds q_last/q_first/q_graph/q_visited
into three HxH matrices A,B,C (precomputed once on device).

v3 strategy (baseline v2 was latency-bound at 184us with every engine
<50% busy, 10 ACT table loads, 63us of HAM throttle):
  - G-packing: per-core 4 batches x G=200 groups are packed into 7 tiles of
    128 rows (per-batch G padded to 224 so batch boundaries inside a tile
    land on 32-aligned partitions, which PE matmul output APs require).
    22% less ACT/DVE/elementwise work than the old 8-tile layout.
  - distances: ONE shared [16,NPAD] fp32r table (rows 0-7: -centered
    coords b-major, rows 8-15: squares); per-tile lhs [16,128] is zero
    except each column's own batch rows (coords via tiny transpose-DMA,
    0.5 in the square rows so the matmul emits 0.5*d2 directly).
    DVE fuses clamp+bias in one tensor_scalar (max then add).
  - visited mask folded as +300 into the distance tile (ds_m), so
    tanh(score - ds_m) = -1 at visited nodes and exp(10*tanh) ~ 0:
    no separate mask add on the softmax path, exp uses scale=10.
  - ACT queue order: all 7 sqrts, then all tanh/exp (tanh+exp share the
    exp_and_others table set) -> exactly 2 table loads.
  - all transposed layouts (emb.T, vis.T) come pre-transposed from the
    host repack; zero on-device DMA transposes of big tiles.
  - f16 everywhere on the score path (as v2), fp32r distances.
Sharding: data-parallel over batch B=32 -> 8 cores x 4 batches.
"""
import sys

sys.path.insert(0, "/opt/trn_rl_repo")

import numpy as np

import concourse.bass as bass
import concourse.tile as tile
from concourse import mybir
from concourse.masks import make_identity


def _ensure_axon_hooks():
    """The image's antenv may lack axon_hooks, which bass_utils imports
    when trace=True under axon. Inject it and register the real NTFF
    profiling hook if the injected .so supports it."""
    try:
        import antenv.axon_hooks  # noqa: F401
        return
    except ImportError:
        pass
    import types
    import antenv

    mod = types.ModuleType("antenv.axon_hooks")
    mod._hook = None
    mod.set_axon_ntff_profile_hook = lambda h: setattr(mod, "_hook", h)
    mod.get_axon_ntff_profile_hook = lambda: mod._hook
    sys.modules["antenv.axon_hooks"] = mod
    antenv.axon_hooks = mod
    try:
        from trn_agent_boot.trn_boot import _ntff_profile_via_ctypes
        mod._hook = _ntff_profile_via_ctypes("/opt/axon/libaxon_pjrt.so")
    except Exception:
        mod._hook = None


_ensure_axon_hooks()

F32 = mybir.dt.float32
F32R = mybir.dt.float32r
F16 = mybir.dt.float16
I32 = mybir.dt.int32

B, N, G, H, NH, D = 32, 2000, 200, 128, 8, 2
NCORES = 8
BPC = B // NCORES          # batches per core
NPAD = 2048                # N padded to 16*128
NCH = NPAD // 128          # n column chunks
GPAD = 224                 # per-batch G padded to a multiple of 32
KH = 72                    # dist-matmul K: 0-7 chat, 32-39 hi, 64-71 lo
GONE = GPAD + 1            # vis.T cols (zero-padded to GPAD) + ones col
NT = BPC * GPAD // 128     # 7 packed g-tiles per core
ALPHA = 1.0 / (NH * np.sqrt(np.float32(H)))   # head-mean * 1/sqrt(H)
AF = mybir.ActivationFunctionType
OP = mybir.AluOpType
W_NAMES = ["Wq_graph", "Wq_first", "Wq_last", "Wq", "W_visited", "Wk"]

# segments: per packed tile, (row0, nrows, batch) pieces; GPAD=224 makes all
# row0 multiples of 32 (PE matmul out base-partition constraint)
def _build_segs():
    segs_all = []
    for t in range(NT):
        segs = []
        r = 0
        while r < 128:
            g_global = 128 * t + r
            b = g_global // GPAD
            seg_len = min(128 - r, GPAD * (b + 1) - g_global)
            segs.append((r, seg_len, b))
            r += seg_len
        segs_all.append(segs)
    return segs_all


SEGS = _build_segs()
# tiles whose last-needed batch b: issue vis(b) before this tile's qsum
TILE_MAX_B = [max(b for (_, _, b) in SEGS[t]) for t in range(NT)]


def _sub_segs(segs):
    """Split segments so each piece is a legal PE matmul output slab:
    base partition 0 (any size) or 32/64/96 with col-tile-size limits."""
    out = []
    for (r0, ln, b) in segs:
        if r0 == 0:
            out.append((0, ln, b))
            continue
        r, rem = r0, ln
        while rem > 0:
            cap = {32: 32, 64: 64, 96: 32}[r]
            take = min(rem, cap)
            out.append((r, take, b))
            r += take
            rem -= take
    return out


SUBSEGS = [_sub_segs(s) for s in SEGS]


def build_nc() -> bass.Bass:
    nc = bass.Bass()

    embc_d = nc.dram_tensor("embc", [BPC, NPAD, H + D], F16,
                            kind="ExternalInput")
    embt_d = nc.dram_tensor("embt", [BPC, H, NPAD], F16, kind="ExternalInput")
    vist_d = nc.dram_tensor("vist", [BPC, NPAD, GONE], F16,
                            kind="ExternalInput")
    gnm_d = nc.dram_tensor("gnm", [NT, 128, NPAD], F16, kind="ExternalInput")
    coordt_d = nc.dram_tensor("coordt", [BPC * D, NPAD], F32,
                              kind="ExternalInput")
    lastn_d = nc.dram_tensor("lastn", [128, NT], I32, kind="ExternalInput")
    lhalf_d = nc.dram_tensor("lhalf", [KH - BPC * D, NT * 128], F16,
                             kind="ExternalInput")
    w6_d = nc.dram_tensor("w6", [H, 6 * H], F32, kind="ExternalInput")
    out_d = nc.dram_tensor("probs", [NT * 128, N], F16, kind="ExternalOutput")

    embc_flat = embc_d.rearrange("b n w -> (b n) w")

    with tile.TileContext(nc) as tc:
        with (
            tc.tile_pool(name="consts", bufs=1) as consts,
            tc.tile_pool(name="inb", bufs=1) as inb,        # big inputs
            tc.tile_pool(name="gpool", bufs=1) as gpool,    # gnm tiles (7)
            tc.tile_pool(name="dsm", bufs=1) as dsmp,       # ds_m tiles (7)
            tc.tile_pool(name="p1s", bufs=2) as p1s,        # phase-1 smalls
            tc.tile_pool(name="csp", bufs=1) as csp,        # coord setup
            tc.tile_pool(name="dsp", bufs=2) as dsp,        # sqrt out
            tc.tile_pool(name="ew", bufs=2) as ew,          # th/e/pr
            tc.tile_pool(name="sm", bufs=2) as sm,          # small scratch
            tc.tile_pool(name="pp_s", bufs=2, space="PSUM") as pp_s,  # 2bk x2
            tc.tile_pool(name="pp_d", bufs=2, space="PSUM") as pp_d,  # 1bk x2
            tc.tile_pool(name="pp_t", bufs=2, space="PSUM") as pp_t,  # 1bk x2
        ):
            # ---- sync queue: small critical loads, then big streams ------
            lastn = consts.tile([128, NT], I32)
            nc.sync.dma_start(out=lastn, in_=lastn_d[:, :])
            ct_raw = csp.tile([BPC * D, NPAD], F32)
            nc.sync.dma_start(out=ct_raw, in_=coordt_d[:, :])
            embn, embt, vist = {}, {}, {}

            def load_embc(b):
                embn[b] = inb.tile([128, NCH, H], F16, tag=f"embn{b}",
                                   name=f"embn{b}")
                nc.sync.dma_start(
                    out=embn[b],
                    in_=embc_d[b, :, 0:H].rearrange("(p c) h -> p c h",
                                                    c=NCH))

            def load_vist(b):
                vist[b] = inb.tile([128, NCH, GONE], F16, tag=f"vist{b}",
                                   name=f"vist{b}")
                nc.sync.dma_start(
                    out=vist[b],
                    in_=vist_d[b].rearrange("(p c) g -> p c g", c=NCH))

            def load_embt(b, eng):
                embt[b] = inb.tile([128, NCH, 128], F16, tag=f"embt{b}",
                                   name=f"embt{b}")
                eng.dma_start(
                    out=embt[b],
                    in_=embt_d[b].rearrange("h (c n) -> h c n", n=128))

            load_embc(0)
            load_vist(0)
            w6 = consts.tile([H, 6 * H], F32)
            nc.sync.dma_start(out=w6, in_=w6_d[:, :])
            load_embc(1)
            load_vist(1)
            load_embt(0, nc.sync)
            load_embc(2)
            load_vist(2)
            load_embt(1, nc.sync)
            load_embc(3)
            load_vist(3)

            # ---- gpsimd queue: iota, lhs static rows, gathers, segs ------
            ident16 = consts.tile([128, 128], F16)
            make_identity(nc, ident16)
            GRP = {0: (0, 2), 1: (2, NT)}
            lhs_g = {}
            for g, (t0, t1) in GRP.items():
                w = (t1 - t0) * 128
                lhs_g[g] = consts.tile([KH, w], F16,
                                       tag=f"lhs_g{g}", name=f"lhs_g{g}")
                nc.gpsimd.memset(lhs_g[g][0:BPC * D], 0.0)
                nc.gpsimd.dma_start(
                    out=lhs_g[g][BPC * D:KH],
                    in_=lhalf_d[:, t0 * 128:t1 * 128])

            def lhs_of(t):
                g = 0 if t < 2 else 1
                o = t * 128 - GRP[g][0] * 128
                return lhs_g[g][:, o:o + 128]

            lne_all = csp.tile([128, NT, H + D], F16)

            def do_gathers(t0, t1):
                for t in range(t0, t1):
                    nc.gpsimd.indirect_dma_start(
                        out=lne_all[:, t, :], out_offset=None, in_=embc_flat,
                        in_offset=bass.IndirectOffsetOnAxis(
                            ap=lastn[:, t:t + 1], axis=0))

            # ---- coord table cs24, built in 512-col chunks across ACT+DVE
            # rows 0-7: chat = -(c-0.5) f16; rows 8-15: f16-hi of chat^2;
            # rows 16-23: f16-lo residual -> the K=24 f16 matmul emits
            # 0.5*d2 - 0.5*r2 exactly to ~1e-7 at f16 speed.
            # table sections at partitions 0 (chat), 32 (hi), 64 (lo) so
            # compute engines can write them (32-aligned bases); the K=72
            # matmul's zero lhs rows make the gap rows inert (zeroed once)
            cs72 = consts.tile([KH, NPAD], F16)
            nc.gpsimd.memset(cs72, 0.0)
            tsq = csp.tile([BPC * D, NPAD], F32)
            thi = csp.tile([BPC * D, NPAD], F16)

            def table_chunk(q):
                sl = slice(q * 512, (q + 1) * 512)
                nc.vector.tensor_scalar(out=cs72[0:BPC * D, sl],
                                        in0=ct_raw[:, sl], scalar1=-1.0,
                                        scalar2=0.5, op0=OP.mult, op1=OP.add)
                nc.scalar.activation(out=tsq[:, sl], in_=cs72[0:BPC * D, sl],
                                     func=AF.Square)
                nc.vector.tensor_copy(out=thi[:, sl], in_=tsq[:, sl])
                nc.scalar.activation(out=cs72[32:32 + BPC * D, sl],
                                     in_=thi[:, sl], func=AF.Copy)
                nc.vector.tensor_tensor(out=cs72[64:64 + BPC * D, sl],
                                        in0=tsq[:, sl], in1=thi[:, sl],
                                        op=OP.subtract)



            # batched last-node prep per group
            lcc_all = p1s.tile([128, NT, D], F16, tag="lcc_all")
            bias_all = consts.tile([128, NT], F32)

            def do_prep(t0, t1):
                nt = t1 - t0
                nc.vector.tensor_scalar(out=lcc_all[:, t0:t1, :],
                                        in0=lne_all[:, t0:t1, H:H + D],
                                        scalar1=-0.5, scalar2=None,
                                        op0=OP.add)
                sq = p1s.tile([128, NT, D], F32, tag="sq_all",
                              name=f"sq_{t0}")
                nc.vector.tensor_tensor(out=sq[:, t0:t1, :],
                                        in0=lcc_all[:, t0:t1, :],
                                        in1=lcc_all[:, t0:t1, :],
                                        op=OP.mult)
                r2 = p1s.tile([128, NT], F32, tag="r2_all", name=f"r2_{t0}")
                nc.vector.tensor_tensor(out=r2[:, t0:t1],
                                        in0=sq[:, t0:t1, 0],
                                        in1=sq[:, t0:t1, 1], op=OP.add)
                nc.vector.tensor_scalar(out=bias_all[:, t0:t1],
                                        in0=r2[:, t0:t1], scalar1=0.5,
                                        scalar2=1e-6, op0=OP.mult,
                                        op1=OP.add)
                lcT_p = pp_t.tile([NT * D, 128], F16, tag="pt",
                                  name=f"lcT_p{t0}")
                nc.tensor.transpose(
                    lcT_p[0:nt * D, :],
                    lcc_all[:, t0:t1, :].rearrange("p t d -> p (t d)"),
                    ident16)
                lcT_s = p1s.tile([NT * D, 128], F16, tag=f"lcT_s{t0}",
                                 name=f"lcT_s{t0}")
                nc.vector.tensor_copy(out=lcT_s[0:nt * D, :],
                                      in_=lcT_p[0:nt * D, :])
                for t in range(t0, t1):
                    for (r0, ln, b) in SEGS[t]:
                        nc.gpsimd.dma_start(
                            out=lhs_of(t)[2 * b:2 * b + 2, r0:r0 + ln],
                            in_=lcT_s[2 * (t - t0):2 * (t - t0) + 2,
                                      r0:r0 + ln])

            do_gathers(0, 2)
            table_chunk(0)
            do_prep(0, 2)
            do_gathers(2, NT)
            table_chunk(1)
            do_prep(2, NT)
            table_chunk(2)
            table_chunk(3)

            # ---- scalar queue: gnm 0-2 early (needed by ds_m t0-2) ------
            gnm = {}

            def load_gnm(t, eng):
                gnm[t] = gpool.tile([128, NPAD], F16, tag=f"gnm{t}",
                                    name=f"gnm{t}")
                eng.dma_start(out=gnm[t], in_=gnm_d[t])

            for t in range(3):
                load_gnm(t, nc.scalar)

            # ---------------- phase 1: dist matmuls + everything else ----
            vembt = consts.tile([H, BPC, GONE], F16)
            qg_s = consts.tile([H, BPC], F32)
            lnetT = consts.tile([H, NT, 128], F16)
            ds_all = {}
            ds_m = {}

            def do_vis(b):
                vemb_p = pp_t.tile([H, GONE], F32, tag="pt", name=f"vemb{b}")
                for c in range(NCH):
                    nc.tensor.matmul(vemb_p, embn[b][:, c, :],
                                     vist[b][:, c, :],
                                     start=(c == 0), stop=(c == NCH - 1))
                nc.vector.tensor_copy(out=vembt[:, b, :], in_=vemb_p)

            def do_dist(t):
                ds = dsp.tile([128, NPAD], F16, tag="ds")
                for q in range(4):
                    d2_p = pp_d.tile([128, 512], F32, tag="pd", name="d2_p")
                    nc.tensor.matmul(d2_p, lhs_of(t),
                                     cs72[:, q * 512:(q + 1) * 512],
                                     start=True, stop=True)
                    nc.scalar.activation(
                        out=ds[:, q * 512:(q + 1) * 512], in_=d2_p,
                        func=AF.Sqrt, bias=bias_all[:, t:t + 1])
                ds_all[t] = ds

            def do_dsm(t):
                dm = dsmp.tile([128, NPAD], F16, tag=f"dsm{t}",
                               name=f"dsm{t}")
                nc.vector.tensor_tensor(out=dm, in0=ds_all[t], in1=gnm[t],
                                        op=OP.add)
                ds_m[t] = dm

            do_dist(0)
            do_vis(0)
            do_dsm(0)
            do_dist(1)
            do_vis(1)
            do_dsm(1)
            load_gnm(3, nc.gpsimd)
            load_gnm(4, nc.gpsimd)
            do_dist(2)
            do_dsm(2)

            # weight chain (tiny, runs inside the sqrt window)
            negi16 = consts.tile([128, 128], F16)
            nc.vector.tensor_scalar(out=negi16, in0=ident16, scalar1=-1.0,
                                    scalar2=None, op0=OP.mult)
            w_s = {n: w6[:, i * H:(i + 1) * H] for i, n in enumerate(W_NAMES)}
            wlf = consts.tile([H, H], F32)
            nc.vector.tensor_tensor(out=wlf, in0=w_s["Wq_last"],
                                    in1=w_s["Wq_first"], op=OP.add)
            mt_p = pp_t.tile([H, H], F32, tag="pt", name="mt_p")
            nc.tensor.matmul(mt_p, w_s["Wq"], w_s["Wk"], start=True, stop=True)
            mt_s = consts.tile([H, H], F32)
            nc.vector.tensor_copy(out=mt_s, in_=mt_p)
            abc = {}
            for nm, lhs, scale in (
                ("A", wlf, ALPHA),
                ("Bm", w_s["Wq_graph"], ALPHA / N),
                ("C", w_s["W_visited"], ALPHA / N),
            ):
                pp = pp_t.tile([H, H], F32, tag="pt", name=f"abc_p_{nm}")
                nc.tensor.matmul(pp, lhs, mt_s, start=True, stop=True)
                abc[nm] = consts.tile([H, H], F16, tag=f"abc_{nm}",
                                      name=f"abc_{nm}")
                nc.vector.tensor_scalar(out=abc[nm], in0=pp,
                                        scalar1=float(scale),
                                        scalar2=None, op0=OP.mult)

            do_dist(3)
            do_vis(2)
            do_dsm(3)
            load_gnm(5, nc.gpsimd)
            load_gnm(6, nc.gpsimd)
            load_embt(2, nc.gpsimd)
            load_embt(3, nc.gpsimd)
            do_dist(4)
            do_vis(3)
            do_dsm(4)
            do_dist(5)
            do_dsm(5)
            do_dist(6)
            do_dsm(6)

            # q_graph per batch + last-node transposes
            for b in range(BPC):
                qg_p = pp_t.tile([H, 1], F32, tag="pt", name=f"qg{b}")
                nc.tensor.matmul(qg_p, abc["Bm"], vembt[:, b, GPAD:GPAD + 1],
                                 start=True, stop=True)
                nc.vector.tensor_copy(out=qg_s[:, b:b + 1], in_=qg_p)
            for t in range(NT):
                lnet_p = pp_t.tile([H, 128], F16, tag="pt", name=f"lnp{t}")
                nc.tensor.transpose(lnet_p, lne_all[:, t, 0:H], ident16)
                nc.vector.tensor_copy(out=lnetT[:, t, :], in_=lnet_p)

            # ---------------- phase 2+3 per tile --------------------------
            qsumt = {}
            for t in range(NT):
                qsum_p = pp_t.tile([H, 128], F32, tag="pt", name=f"qsp{t}")
                nc.tensor.matmul(qsum_p, abc["A"], lnetT[:, t, :],
                                 start=True, stop=False)
                for i, (r0, ln, b) in enumerate(SEGS[t]):
                    gg = 128 * t + r0 - GPAD * b
                    nc.tensor.matmul(qsum_p[:, r0:r0 + ln], abc["C"],
                                     vembt[:, b, gg:gg + ln],
                                     start=False, stop=(i == len(SEGS[t]) - 1))
                qt = sm.tile([H, 128], F16, tag="qsumt", name=f"qsumt{t}")
                for (r0, ln, b) in SEGS[t]:
                    nc.vector.tensor_scalar(out=qt[:, r0:r0 + ln],
                                            in0=qsum_p[:, r0:r0 + ln],
                                            scalar1=qg_s[:, b:b + 1],
                                            scalar2=None, op0=OP.add)
                qsumt[t] = qt

                th = ew.tile([128, NPAD], F16, tag="th")
                sps = []
                for hw in range(2):
                    sp = pp_s.tile([128, 1024], F32, tag="ps", name="sp")
                    sps.append(sp)
                    for q in range(2):
                        o = hw * 1024 + q * 512
                        nc.tensor.matmul(
                            sp[:, q * 512:(q + 1) * 512], negi16,
                            ds_m[t][:, o:o + 512],
                            start=True, stop=False)
                embt2 = {b: embt[b].rearrange("p c n -> p (c n)")
                         for b in set(b for (_, _, b) in SEGS[t])}
                for i, (r0, ln, b) in enumerate(SUBSEGS[t]):
                    last = i == len(SUBSEGS[t]) - 1
                    tp = None if r0 == 0 else (0, r0)
                    for hw in range(2):
                        for q in range(2):
                            o = hw * 1024 + q * 512
                            nc.tensor.matmul(
                                sps[hw][r0:r0 + ln, q * 512:(q + 1) * 512],
                                qsumt[t][:, r0:r0 + ln],
                                embt2[b][:, o:o + 512],
                                start=False, stop=last, tile_position=tp)
                for hw in range(2):
                    nc.scalar.activation(
                        out=th[:, hw * 1024:(hw + 1) * 1024], in_=sps[hw],
                        func=AF.Tanh)
                e = ew.tile([128, NPAD], F16, tag="e")
                esum = sm.tile([128, 1], F32, tag="esum")
                nc.scalar.activation(out=e, in_=th, func=AF.Exp, scale=10.0,
                                     accum_out=esum[:, :])
                nc.vector.reciprocal(out=esum, in_=esum)
                pr = ew.tile([128, NPAD], F16, tag="pr")
                nc.vector.tensor_scalar(out=pr, in0=e, scalar1=esum[:, :],
                                        scalar2=None, op0=OP.mult)
                nc.sync.dma_start(out=out_d[t * 128:(t + 1) * 128, :],
                                  in_=pr[:, 0:N])
    return nc


def _split_multi_waits(bir: bytes, max_inline: int = 1) -> bytes:
    """This walrus build only accepts one inline sync-wait per instruction;
    Tile inlines many. Split extras into standalone EventSemaphore waits
    (same engine, immediately before), which is exactly the raw-bass form."""
    import orjson

    j = orjson.loads(bir)
    ctr = 0
    for fn in j["functions"]:
        for blk in fn["blocks"]:
            insts = blk.get("instructions")
            if not insts:
                continue
            out = []
            for inst in insts:
                si = inst.get("sync_info")
                waits = (si or {}).get("on_wait") or []
                if len(waits) > max_inline:
                    for w in waits[:-max_inline]:
                        ctr += 1
                        out.append({
                            "name": f"SW-{ctr}",
                            "opcode": "EventSemaphore",
                            "engine": inst["engine"],
                            "ins": [],
                            "outs": [],
                            "sync_info": {"on_wait": [w], "on_update": []},
                        })
                    si["on_wait"] = waits[-max_inline:]
                out.append(inst)
            blk["instructions"] = out
    return orjson.dumps(j)


_NC = None


def _get_nc():
    global _NC
    if _NC is None:
        _NC = build_nc()
        transformed = _split_multi_waits(_NC.to_json_bytes())
        _NC.to_json_bytes = lambda: transformed
    return _NC


def make_in_maps(embeddings, coordinates, last_node, group_ninf_mask,
                 Wq_graph, Wq_first, Wq_last, Wq, W_visited, Wk):
    """Shard + pad + repack full inputs into 8 per-core input maps."""
    emb_p = np.zeros((B, NPAD, H + D), np.float16)
    emb_p[:, :N, :H] = embeddings.astype(np.float16)
    emb_p[:, :N, H:] = coordinates.astype(np.float16)
    embt_p = np.ascontiguousarray(
        emb_p[:, :, :H].transpose(0, 2, 1))  # (B, H, NPAD)
    coord_p = np.zeros((B, NPAD, D), np.float32)
    coord_p[:, :N] = coordinates
    # (B, D, NPAD) -> per core [8, NPAD] -> pre-tiled [128, NPAD/16]
    coordt = np.ascontiguousarray(coord_p.transpose(0, 2, 1))

    vis = np.isneginf(group_ninf_mask)                         # (B, G, N)
    vist_p = np.zeros((B, NPAD, GONE), np.float16)
    vist_p[:, :N, :G] = vis.transpose(0, 2, 1)
    vist_p[:, :, GPAD] = 1.0                                      # colsum ones

    gnm300 = np.full((B, GPAD, NPAD), 300.0, np.float16)
    gnm300[:, :G, :N] = vis.astype(np.float16) * np.float16(300.0)

    lastn = np.zeros((B, GPAD), np.int32)
    lastn[:, :G] = np.asarray(last_node).astype(np.int64).astype(np.int32)
    # pre-offset indices into the per-core flattened (BPC*NPAD, .) source
    lastn += (np.arange(B, dtype=np.int32) % BPC)[:, None] * NPAD

    # static 0.5 pattern for the lhs square rows (rows 8-23 of lhs_all):
    # row r (0..15) -> lhs row 8+r, batch (r % 8)//2; 0.5 where the packed
    # g-column belongs to that batch
    col_b = np.arange(NT * 128) // GPAD
    lhalf = np.zeros((KH - BPC * D, NT * 128), np.float16)
    for sec in (32, 64):                    # hi and lo 0.5-pattern rows
        for b in range(BPC):
            for dd in range(D):
                lhalf[sec + 2 * b + dd - BPC * D,
                      col_b == b] = 0.5

    w6 = np.ascontiguousarray(np.concatenate(
        [np.asarray(w, np.float32) for w in
         (Wq_graph, Wq_first, Wq_last, Wq, W_visited, Wk)], axis=1))

    in_maps = []
    for i in range(NCORES):
        sl = slice(i * BPC, (i + 1) * BPC)
        lpk = lastn[sl].reshape(NT, 128)          # packed rows, tile-major
        m = {
            "embc": np.ascontiguousarray(emb_p[sl]),
            "embt": np.ascontiguousarray(embt_p[sl]),
            "vist": np.ascontiguousarray(vist_p[sl]),
            "gnm": np.ascontiguousarray(
                gnm300[sl].reshape(NT, 128, NPAD)),
            "coordt": np.ascontiguousarray(
                coordt[sl].reshape(BPC * D, NPAD)),
            "lastn": np.ascontiguousarray(lpk.T),  # [128, NT]
            "lhalf": lhalf,
            "w6": w6,
        }
        in_maps.append(m)
    return in_maps


def kernel(embeddings, coordinates, last_node, group_ninf_mask, S,
           Wq_graph, Wq_first, Wq_last, Wq, W_visited, Wk, **run_kwargs):
    from concourse.bass_utils import run_bass_kernel_spmd

    nc = _get_nc()
    in_maps = make_in_maps(
        np.asarray(embeddings), np.asarray(coordinates), np.asarray(last_node),
        np.asarray(group_ninf_mask), np.asarray(Wq_graph), np.asarray(Wq_first),
        np.asarray(Wq_last), np.asarray(Wq), np.asarray(W_visited),
        np.asarray(Wk))
    res = run_bass_kernel_spmd(nc, in_maps, core_ids=list(range(NCORES)),
                               **run_kwargs)
    # unpack: [NT*128, N] -> (BPC, GPAD, N) -> (BPC, G, N)
    outs = []
    for r in res.results:
        o = r["probs"].reshape(BPC, GPAD, N)[:, :G, :]
        outs.append(o)
    out = np.concatenate(outs, axis=0)
    kernel.last_results = res
    return out.astype(np.float32)
